# revision 1
# baseline (speedup 1.0000x reference)
"""Causal self-attention (B=4, T=2048, C=1024, H=16) on 8 trn2 NeuronCores.

Sharding: head-pair parallel. Core c owns heads {2c, 2c+1} for all 4 batches.
 - host: x is pre-transposed to xT [C, B*T]; W_qkv is pre-sliced per core into
   wq/wk/wv [C, 128] (2 heads x 64, softmax scale folded into wq), W_proj and
   biases broadcast.
 - device per core: qkv projections as fp32r matmuls producing qT/kT [d2, T]
   (d on partitions) and vT [d2, T]; vT is PE-transposed per 128-tile into
   v [T, 64]-per-head tiles with an appended ones column.
 - attention in S^T orientation: S^T[tk, tq] = kT.T@qT tiles [128, 512] with the
   causal mask preloaded into PSUM via an identity matmul; softmax without
   max-subtraction (|S| <= ~20, safe in fp32): P^T = exp(S^T) on ScalarE
   (PSUM->SBUF, rounded to f32r). O-matmul lhsT = [v_h | ones] (M=65) yields
   both O^T[d, tq] and the denominator row l in one pass. Normalize via
   reciprocal + K=1 broadcast matmul + DVE multiply.
 - per-batch AllToAll (1 MB/rank) reshards O^T from head-shards to
   token-shards; column-parallel out-projection with fused bias produces
   out^T [C, 1024 tokens] per core; host reassembles.
"""
import numpy as np
import concourse.bacc as bacc
import concourse.mybir as mybir
import concourse.tile as tile
from concourse.bass_utils import run_bass_kernel_spmd
from concourse.masks import make_identity

F32 = mybir.dt.float32
F32R = mybir.dt.float32r
Exp = mybir.ActivationFunctionType.Exp

NCORES = 8
B, T, C, H = 4, 2048, 1024, 16
HD = C // H          # 64
HL = H // NCORES     # 2 heads per core
D2 = HL * HD         # 128 rows of local head-pair dims
TB = T               # tokens per batch
NKC = C // 128       # 8 contraction chunks
NCH = TB // 512      # 4 tq chunks per batch
NTK = TB // 128      # 16 tk tiles per batch
PIECE = TB // NCORES  # 256 tokens per (batch, core) piece after AllToAll

_CACHE = {}


def _build(sim=False):
    nc = bacc.Bacc("TRN2", target_bir_lowering=False, debug=False,
                   num_devices=1 if sim else NCORES)
    xt = nc.dram_tensor("xt", [C, B * T], F32R, kind="ExternalInput").ap()
    wq = nc.dram_tensor("wq", [C, D2], F32R, kind="ExternalInput").ap()
    wk = nc.dram_tensor("wk", [C, D2], F32R, kind="ExternalInput").ap()
    wv = nc.dram_tensor("wv", [C, D2], F32R, kind="ExternalInput").ap()
    wp = nc.dram_tensor("wp", [C, C], F32R, kind="ExternalInput").ap()
    bqkv = nc.dram_tensor("bqkv", [D2, 3], F32, kind="ExternalInput").ap()
    bp = nc.dram_tensor("bp", [128, NKC], F32, kind="ExternalInput").ap()
    outp = nc.dram_tensor("outp", [C, B * PIECE], F32, kind="ExternalOutput").ap()

    inb = [nc.dram_tensor(f"inb{b}", [NCORES, D2, PIECE], F32R) for b in range(B)]
    outb = [nc.dram_tensor(f"outb{b}", [NCORES, D2, PIECE], F32R) for b in range(B)]

    with tile.TileContext(nc) as tc:
        with (
            tc.tile_pool(name="const", bufs=1) as cpool,
            tc.tile_pool(name="w", bufs=1) as wpool,
            tc.tile_pool(name="xt", bufs=16) as xpool,
            tc.tile_pool(name="qk", bufs=2) as qkpool,
            tc.tile_pool(name="vstg", bufs=1) as vstgpool,
            tc.tile_pool(name="vh", bufs=2) as vhpool,
            tc.tile_pool(name="pt", bufs=5) as ptpool,
            tc.tile_pool(name="small", bufs=3) as smallpool,
            tc.tile_pool(name="ofin", bufs=4) as ofinpool,
            tc.tile_pool(name="proj", bufs=3) as projpool,
            tc.tile_pool(name="otp", bufs=9) as otpool,
            tc.tile_pool(name="mm", bufs=2, space="PSUM") as mmps,
            tc.tile_pool(name="s", bufs=2, space="PSUM") as sps,
            tc.tile_pool(name="o", bufs=1, space="PSUM") as ops,
            
        ):
            # ---- constants ----
            ident32 = cpool.tile([128, 128], F32)
            make_identity(nc, ident32[:])
            idr = cpool.tile([128, 128], F32R)
            mask32 = cpool.tile([128, 512], F32)
            masks = cpool.tile([128, 4 * 512], F32R)
            ones32 = cpool.tile([128, 16], F32)
            ones64 = cpool.tile([1, 64], F32)
            onesr = cpool.tile([1, 64], F32R)
            nc.gpsimd.memset(ones32[:], 1.0)
            nc.gpsimd.memset(ones64[:], 1.0)
            with nc.allow_low_precision(reason="f32r operand staging"):
                nc.vector.tensor_copy(idr[:], ident32[:])
                nc.vector.tensor_copy(onesr[:], ones64[:])
                for m in range(4):
                    nc.gpsimd.memset(mask32[:], 0.0)
                    # keep where tq_local >= tk_local + 128*m
                    nc.gpsimd.affine_select(
                        out=mask32[:], in_=mask32[:],
                        compare_op=mybir.AluOpType.is_ge, fill=-1e30,
                        base=-128 * m, channel_multiplier=-1,
                        pattern=[[1, 512]],
                    )
                    nc.vector.tensor_copy(masks[:, 512 * m:512 * (m + 1)],
                                          mask32[:])

            # ---- weights ----
            wq_sb = wpool.tile([128, NKC, D2], F32R)
            wk_sb = wpool.tile([128, NKC, D2], F32R)
            wv_sb = wpool.tile([128, NKC, D2], F32R)
            for t, d in ((wq_sb, wq), (wk_sb, wk), (wv_sb, wv)):
                nc.sync.dma_start(
                    t[:], d.rearrange("(kc p) m -> p kc m", p=128))
            wp_sb = wpool.tile([128, NKC, C], F32R)
            nc.sync.dma_start(
                wp_sb[:], wp.rearrange("(kc p) m -> p kc m", p=128))
            bqkv_sb = cpool.tile([D2, 3], F32)
            nc.sync.dma_start(bqkv_sb[:], bqkv)
            bp_sb = cpool.tile([128, NKC], F32)
            nc.sync.dma_start(bp_sb[:], bp)

            for b in range(B):
                g0 = b * TB
                # ---- qkv projections ----
                qT = qkpool.tile([D2, TB], F32R, tag="qT")
                kT = qkpool.tile([D2, TB], F32R, tag="kT")
                vT = vstgpool.tile([D2, TB], F32)
                for n in range(NCH):
                    xts = []
                    for kc in range(NKC):
                        xtile = xpool.tile([128, 512], F32R)
                        nc.sync.dma_start(
                            xtile[:],
                            xt[128 * kc:128 * (kc + 1),
                               g0 + 512 * n:g0 + 512 * (n + 1)])
                        xts.append(xtile)
                    for w_sb, col in ((wq_sb, 0), (wk_sb, 1), (wv_sb, 2)):
                        ps = mmps.tile([128, 512], F32, tag="ps")
                        for kc in range(NKC):
                            nc.tensor.matmul(
                                ps[:], w_sb[:, kc, :],
                                xts[kc][:], start=(kc == 0),
                                stop=(kc == NKC - 1))
                        dst = (qT, kT, vT)[col]
                        with nc.allow_low_precision(reason="f32r qkv"):
                            nc.vector.tensor_scalar_add(
                                dst[:, 512 * n:512 * (n + 1)], ps[:],
                                bqkv_sb[:, col:col + 1])

                # ---- v transposes: vT [d2, T] -> per-head v [T, 65] tiles ----
                vh = [vhpool.tile([128, NTK * 65], F32R, tag=f"vh{h}",
                                  name=f"vh{h}") for h in range(HL)]
                for h in range(HL):
                    with nc.allow_low_precision(reason="f32r v ones"):
                        nc.vector.tensor_copy(vh[h][:, 64::65], ones32[:])
                    for tk in range(NTK):
                        vt_ps = mmps.tile([128, 64], F32, tag="ps", name="vt_ps")
                        nc.tensor.transpose(
                            vt_ps[:],
                            vT[64 * h:64 * (h + 1), 128 * tk:128 * (tk + 1)],
                            ident32[64 * h:64 * (h + 1), 64 * h:64 * (h + 1)])
                        with nc.allow_low_precision(reason="f32r v"):
                            nc.vector.tensor_copy(
                                vh[h][:, 65 * tk:65 * tk + 64], vt_ps[:])

                # ---- attention per tq-chunk ----
                for j in range(NCH):
                    o_ps = [ops.tile([65, 512], F32, tag=f"o{h}", name=f"o{h}")
                            for h in range(HL)]
                    ktop = 4 * j + 4
                    for tk in range(ktop):
                        m = tk - 4 * j
                        # cols [0, z) of this tile are fully causal-masked
                        z = 128 * m if m > 0 else 0
                        w = 512 - z
                        s_ps = sps.tile([128, 1024], F32, tag="s_ps")
                        if m >= 0:
                            for h in range(HL):
                                nc.tensor.matmul(
                                    s_ps[:, 512 * h + z:512 * (h + 1)],
                                    idr[:],
                                    masks[:, 512 * m + z:512 * (m + 1)],
                                    start=True, stop=False)
                        # K=64 pair at row groups (0,0)/(64,0) -> concurrent
                        for h in range(HL):
                            nc.tensor.matmul(
                                s_ps[:, 512 * h + z:512 * (h + 1)],
                                kT[64 * h:64 * (h + 1),
                                   128 * tk:128 * (tk + 1)],
                                qT[64 * h:64 * (h + 1),
                                   512 * j + z:512 * (j + 1)],
                                start=(m < 0), stop=True)
                        pt = ptpool.tile([128, 1024], F32R, tag="pt")
                        if z:
                            exp_src = s_ps[:].rearrange(
                                "p (g c) -> p g c", g=2)[:, :, z:]
                            exp_dst = pt[:].rearrange(
                                "p (g c) -> p g c", g=2)[:, :, z:]
                            nc.scalar.activation(exp_dst, exp_src, Exp)
                        else:
                            nc.scalar.activation(pt[:], s_ps[:], Exp)
                        for h in range(HL):
                            nc.tensor.matmul(
                                o_ps[h][0:65, z:512],
                                vh[h][:, 65 * tk:65 * (tk + 1)],
                                pt[:, 512 * h + z:512 * (h + 1)],
                                start=(tk == 0), stop=(tk == ktop - 1))
                    for h in range(HL):
                        o_sb = smallpool.tile([65, 512], F32, tag="osb2")
                        nc.vector.tensor_copy(o_sb[:], o_ps[h][:])
                        r_sb = smallpool.tile([1, 512], F32R, tag="r")
                        with nc.allow_low_precision(reason="softmax denom"):
                            nc.vector.reciprocal(r_sb[:], o_sb[64:65, :])
                        rb_ps = mmps.tile([64, 512], F32, tag="ps", name="rb_ps")
                        nc.tensor.matmul(rb_ps[:], onesr[:], r_sb[:],
                                         start=True, stop=True)
                        rb_sb = smallpool.tile([64, 512], F32, tag="rb")
                        nc.vector.tensor_copy(rb_sb[:], rb_ps[:])
                        ofin = ofinpool.tile([64, 512], F32R)
                        with nc.allow_low_precision(reason="f32r O"):
                            nc.gpsimd.tensor_mul(ofin[:], o_sb[0:64, :],
                                                 rb_sb[:])
                        for half in range(2):
                            s8 = 2 * j + half
                            nc.sync.dma_start(
                                inb[b].ap()[s8, 64 * h:64 * (h + 1), :],
                                ofin[:, 256 * half:256 * (half + 1)])

                # ---- AllToAll: head-shards -> token-shards ----
                if sim:
                    # stand-in with comparable cost for the cost-model sim
                    nc.sync.dma_start(outb[b].ap(), inb[b].ap())
                else:
                    nc.gpsimd.collective_compute(
                        "AllToAll", mybir.AluOpType.bypass,
                        replica_groups=[list(range(NCORES))],
                        ins=[inb[b].ap().opt()], outs=[outb[b].ap().opt()],
                    )

                # ---- out projection (column-parallel, out^T) ----
                ots = []
                for s8 in range(NCORES):
                    ot = otpool.tile([128, PIECE], F32R, tag="ot")
                    nc.sync.dma_start(ot[:], outb[b].ap()[s8])
                    ots.append(ot)
                for mcol in range(NKC):
                    pp = mmps.tile([128, PIECE], F32, tag="ps")
                    for s8 in range(NCORES):
                        nc.tensor.matmul(
                            pp[:],
                            wp_sb[:, s8, 128 * mcol:128 * (mcol + 1)],
                            ots[s8][:], start=(s8 == 0),
                            stop=(s8 == NCORES - 1))
                    osb = projpool.tile([128, PIECE], F32, tag="osb")
                    nc.vector.tensor_scalar_add(osb[:], pp[:],
                                                bp_sb[:, mcol:mcol + 1])
                    nc.sync.dma_start(
                        outp[128 * mcol:128 * (mcol + 1),
                             PIECE * b:PIECE * (b + 1)], osb[:])
    nc.compile()
    return nc


def _get_nc():
    if "nc" not in _CACHE:
        _CACHE["nc"] = _build()
    return _CACHE["nc"]


def kernel(x, W_qkv, b_qkv, W_proj, b_proj):
    x = np.asarray(x, dtype=np.float32)
    W_qkv = np.asarray(W_qkv, dtype=np.float32)
    b_qkv = np.asarray(b_qkv, dtype=np.float32)
    W_proj = np.asarray(W_proj, dtype=np.float32)
    b_proj = np.asarray(b_proj, dtype=np.float32)

    scale = 1.0 / np.sqrt(HD)
    xt = np.ascontiguousarray(x.reshape(B * T, C).T)          # [C, B*T]
    wp = np.ascontiguousarray(W_proj)                          # [C, C]
    bp = np.ascontiguousarray(b_proj.reshape(NKC, 128).T)      # [128, 8]

    qw = W_qkv[:, 0:C]
    kw = W_qkv[:, C:2 * C]
    vw = W_qkv[:, 2 * C:3 * C]
    qb, kb, vb = b_qkv[0:C], b_qkv[C:2 * C], b_qkv[2 * C:3 * C]

    in_maps = []
    for c in range(NCORES):
        cols = slice(2 * c * HD, (2 * c + 2) * HD)  # this core's 128 dims
        bq = np.stack([qb[cols] * scale, kb[cols], vb[cols]], axis=1)  # [128,3]
        in_maps.append({
            "xt": xt,
            "wq": np.ascontiguousarray(qw[:, cols] * scale),
            "wk": np.ascontiguousarray(kw[:, cols]),
            "wv": np.ascontiguousarray(vw[:, cols]),
            "wp": wp,
            "bqkv": np.ascontiguousarray(bq),
            "bp": bp,
        })

    nc = _get_nc()
    _CACHE["last_in_maps"] = in_maps
    res = run_bass_kernel_spmd(nc, in_maps, core_ids=list(range(NCORES)))

    # outp[c]: [C, B*PIECE] (cols: b-major, then 256 tokens of piece c)
    allo = np.stack([res.results[c]["outp"] for c in range(NCORES)])
    allo = allo.reshape(NCORES, C, B, PIECE)       # [c, ch, b, u]
    out = allo.transpose(2, 0, 3, 1).reshape(B, T, C)
    return np.ascontiguousarray(out)



# revision 6
# speedup vs baseline: 1.0314x; 1.0314x over previous
"""Causal self-attention (B=4, T=2048, C=1024, H=16) on 8 trn2 NeuronCores.

Sharding: head-pair parallel. Core c owns heads {2c, 2c+1} for all 4 batches.
 - host: x is pre-transposed to xT [C, B*T] in bf16; W_qkv pre-sliced per core
   into wq/wk/wv [C, 128] bf16 (2 heads x 64, softmax scale folded into wq),
   W_proj bf16 and f32 biases broadcast.
 - device per core: qkv projections as bf16 matmuls producing qT/kT/vT
   [d2, T] bf16 (d on partitions); vT is PE-transposed per 128-tile (both
   heads in one [128,128] transpose) into per-head v [T, 64] tiles with an
   appended ones column (vh).
 - attention in S^T orientation: S^T[tk, tq] = kT.T@qT tiles [128, 512].
   No mask matmul: P^T = exp(S^T) on ScalarE (PSUM->SBUF bf16), then the
   128x128 diagonal wedge of P^T is zeroed on GpSimd (affine_select fill=0);
   exp of unmasked future logits is finite so this is safe. Diagonal tiles
   are processed first within each tq chunk so the m=0 full-width tile opens
   the O accumulation. O-matmul lhsT = [v_h | ones] (M=65) yields O^T and the
   softmax denominator row in one pass. Normalize via reciprocal + K=1
   broadcast matmul + GpSimd multiply (both heads batched per chunk).
 - per-batch AllToAll (512 KB/rank, bf16) reshards O^T from head-shards to
   token-shards; qkv of batch b+1 is issued between the AllToAll and the
   out-projection of batch b so the PE never idles on the collective.
   Column-parallel out-projection with fused bias produces out^T bf16;
   host reassembles and upcasts.
"""
import numpy as np
import ml_dtypes
import concourse.bacc as bacc
import concourse.mybir as mybir
import concourse.tile as tile
from concourse.bass_utils import run_bass_kernel_spmd
from concourse.masks import make_identity

F32 = mybir.dt.float32
F32R = mybir.dt.float32r
BF16 = mybir.dt.bfloat16
Exp = mybir.ActivationFunctionType.Exp

NCORES = 8
B, T, C, H = 4, 2048, 1024, 16
HD = C // H          # 64
HL = H // NCORES     # 2 heads per core
D2 = HL * HD         # 128 rows of local head-pair dims
TB = T               # tokens per batch
NKC = C // 128       # 8 contraction chunks
NCH = TB // 512      # 4 tq chunks per batch
NTK = TB // 128      # 16 tk tiles per batch
PIECE = TB // NCORES  # 256 tokens per (batch, core) piece after AllToAll

_CACHE = {}


def _build(sim=False):
    nc = bacc.Bacc("TRN2", target_bir_lowering=False, debug=False,
                   num_devices=1 if sim else NCORES)
    xt = nc.dram_tensor("xt", [C, B * T], BF16, kind="ExternalInput").ap()
    wq = nc.dram_tensor("wq", [C, D2], BF16, kind="ExternalInput").ap()
    wk = nc.dram_tensor("wk", [C, D2], BF16, kind="ExternalInput").ap()
    wv = nc.dram_tensor("wv", [C, D2], BF16, kind="ExternalInput").ap()
    wp = nc.dram_tensor("wp", [C, C], BF16, kind="ExternalInput").ap()
    bqkv = nc.dram_tensor("bqkv", [D2, 3], F32, kind="ExternalInput").ap()
    bp = nc.dram_tensor("bp", [128, NKC], F32, kind="ExternalInput").ap()
    outp = nc.dram_tensor("outp", [C, B * PIECE], BF16,
                          kind="ExternalOutput").ap()

    inb = [nc.dram_tensor(f"inb{b}", [NCORES, D2, PIECE], BF16)
           for b in range(B)]
    outb = [nc.dram_tensor(f"outb{b}", [NCORES, D2, PIECE], BF16)
            for b in range(B)]

    with tile.TileContext(nc) as tc:
        with (
            tc.tile_pool(name="const", bufs=1) as cpool,
            tc.tile_pool(name="w", bufs=1) as wpool,
            tc.tile_pool(name="xt", bufs=16) as xpool,
            tc.tile_pool(name="qk", bufs=2) as qkpool,
            tc.tile_pool(name="vstg", bufs=1) as vstgpool,
            tc.tile_pool(name="vh", bufs=2) as vhpool,
            tc.tile_pool(name="pt", bufs=5) as ptpool,
            tc.tile_pool(name="small", bufs=3) as smallpool,
            tc.tile_pool(name="ofin", bufs=3) as ofinpool,
            tc.tile_pool(name="proj", bufs=3) as projpool,
            tc.tile_pool(name="otp", bufs=9) as otpool,
            tc.tile_pool(name="mm", bufs=2, space="PSUM") as mmps,
            tc.tile_pool(name="s", bufs=2, space="PSUM") as sps,
            tc.tile_pool(name="o", bufs=1, space="PSUM") as ops,
        ):
            # ---- constants ----
            identb = cpool.tile([128, 128], BF16)
            make_identity(nc, identb[:])
            ones32 = cpool.tile([128, 32], F32)
            onesb = cpool.tile([128, 32], BF16)
            ones64 = cpool.tile([1, 64], F32)
            onesr = cpool.tile([1, 64], F32R)
            nc.gpsimd.memset(ones32[:], 1.0)
            nc.gpsimd.memset(ones64[:], 1.0)
            with nc.allow_low_precision(reason="low precision staging"):
                nc.vector.tensor_copy(onesb[:], ones32[:])
                nc.vector.tensor_copy(onesr[:], ones64[:])

            # ---- weights (wp is chunk-loaded inside emit_qkv(0)) ----
            wq_sb = wpool.tile([128, NKC, D2], BF16)
            wk_sb = wpool.tile([128, NKC, D2], BF16)
            wv_sb = wpool.tile([128, NKC, D2], BF16)
            for t, d in ((wq_sb, wq), (wk_sb, wk), (wv_sb, wv)):
                nc.sync.dma_start(
                    t[:], d.rearrange("(kc p) m -> p kc m", p=128))
            wp_sb = wpool.tile([128, NKC, C], BF16)
            bqkv_sb = cpool.tile([D2, 3], F32)
            nc.sync.dma_start(bqkv_sb[:], bqkv)
            bp_sb = cpool.tile([128, NKC], F32)
            nc.sync.dma_start(bp_sb[:], bp)

            qT = {}
            kT = {}
            vh = {}

            def emit_qkv(b):
                g0 = b * TB
                qT[b] = qkpool.tile([D2, TB], BF16, tag="qT", name="qT")
                kT[b] = qkpool.tile([D2, TB], BF16, tag="kT", name="kT")
                vT = vstgpool.tile([D2, TB], BF16)
                for n in range(NCH):
                    xts = []
                    for kc in range(NKC):
                        xtile = xpool.tile([128, 512], BF16)
                        nc.sync.dma_start(
                            xtile[:],
                            xt[128 * kc:128 * (kc + 1),
                               g0 + 512 * n:g0 + 512 * (n + 1)])
                        xts.append(xtile)
                    if b == 0:
                        # interleave W_proj chunk loads behind batch-0 x
                        for kc2 in (2 * n, 2 * n + 1):
                            nc.sync.dma_start(
                                wp_sb[:, kc2, :],
                                wp[128 * kc2:128 * (kc2 + 1), :])
                    for w_sb, col, dst in ((wq_sb, 0, qT[b]),
                                           (wk_sb, 1, kT[b]),
                                           (wv_sb, 2, vT)):
                        ps = sps.tile([128, 1024], F32, tag="s_ps",
                                      name="ps")
                        for kc in range(NKC):
                            nc.tensor.matmul(
                                ps[:, 0:512], w_sb[:, kc, :],
                                xts[kc][:], start=(kc == 0),
                                stop=(kc == NKC - 1))
                        with nc.allow_low_precision(reason="bf16 qkv"):
                            nc.vector.tensor_scalar_add(
                                dst[:, 512 * n:512 * (n + 1)], ps[:, 0:512],
                                bqkv_sb[:, col:col + 1])

                # ---- v transpose: vT [d2, T] -> vh [tk, d] per head + ones
                # layout: vh[:, h*(NTK*65) + tk*65 + (0:64)] = v_h tile tk
                vh[b] = vhpool.tile([128, HL * NTK * 65], BF16, tag="vh", name="vh")
                with nc.allow_low_precision(reason="bf16 v ones"):
                    nc.vector.tensor_copy(vh[b][:, 64::65], onesb[:])
                for tk0 in range(0, NTK, 4):
                    vt_ps = mmps.tile([128, 512], BF16, tag="vtps")
                    for i in range(4):
                        nc.tensor.transpose(
                            vt_ps[:, 128 * i:128 * (i + 1)],
                            vT[:, 128 * (tk0 + i):128 * (tk0 + i + 1)],
                            identb[:])
                    with nc.allow_low_precision(reason="bf16 v"):
                        nc.vector.tensor_copy(
                            vh[b][:].rearrange(
                                "p (h tk c) -> p h tk c",
                                h=HL, tk=NTK)[:, :, tk0:tk0 + 4, 0:64],
                            vt_ps[:].rearrange(
                                "p (i h c) -> p h i c", i=4, h=HL))

            def emit_attn(b):
                for j in range(NCH):
                    o_ps = ops.tile([65, 1024], F32, tag="o")
                    # diagonal tiles first (m=0 is full width and opens the
                    # O accumulation); then the full tiles below the diagonal
                    seq = list(range(4 * j, 4 * j + 4)) + list(range(4 * j))
                    for idx, tk in enumerate(seq):
                        m = tk - 4 * j
                        z = 128 * m if m >= 0 else 0
                        s_ps = sps.tile([128, 1024], F32, tag="s_ps")
                        for h in range(HL):
                            nc.tensor.matmul(
                                s_ps[:, 512 * h + z:512 * (h + 1)],
                                kT[b][64 * h:64 * (h + 1),
                                      128 * tk:128 * (tk + 1)],
                                qT[b][64 * h:64 * (h + 1),
                                      512 * j + z:512 * (j + 1)],
                                start=True, stop=True)
                        pt = ptpool.tile([128, 1024], BF16, tag="pt")
                        if z:
                            exp_src = s_ps[:].rearrange(
                                "p (g c) -> p g c", g=2)[:, :, z:]
                            exp_dst = pt[:].rearrange(
                                "p (g c) -> p g c", g=2)[:, :, z:]
                            nc.scalar.activation(exp_dst, exp_src, Exp)
                        else:
                            nc.scalar.activation(pt[:], s_ps[:], Exp)
                        if m >= 0:
                            # zero the causal wedge: keep where tq_l >= tk_l
                            nc.gpsimd.affine_select(
                                out=pt[:].rearrange(
                                    "p (g c) -> p g c", g=2)[:, :, z:z + 128],
                                in_=pt[:].rearrange(
                                    "p (g c) -> p g c", g=2)[:, :, z:z + 128],
                                compare_op=mybir.AluOpType.is_ge, fill=0.0,
                                base=0, channel_multiplier=-1,
                                pattern=[[0, HL], [1, 128]],
                            )
                        for h in range(HL):
                            nc.tensor.matmul(
                                o_ps[0:65, 512 * h + z:512 * (h + 1)],
                                vh[b][:, (h * NTK + tk) * 65:
                                      (h * NTK + tk + 1) * 65],
                                pt[:, 512 * h + z:512 * (h + 1)],
                                start=(idx == 0), stop=(idx == len(seq) - 1))

                    # ---- normalize (both heads batched) ----
                    o_sb = smallpool.tile([65, 1024], F32, tag="osb2")
                    nc.vector.tensor_copy(o_sb[:], o_ps[:])
                    r_sb = smallpool.tile([1, 1024], F32R, tag="r")
                    with nc.allow_low_precision(reason="softmax denom"):
                        nc.vector.reciprocal(r_sb[:], o_sb[64:65, :])
                    rb_ps = sps.tile([128, 1024], F32, tag="s_ps")
                    nc.tensor.matmul(rb_ps[0:64, :], onesr[:, 0:64], r_sb[:],
                                     start=True, stop=True)
                    rb_sb = smallpool.tile([64, 1024], F32, tag="rb")
                    nc.vector.tensor_copy(rb_sb[:], rb_ps[0:64, :])
                    ofin = ofinpool.tile([64, 1024], BF16)
                    with nc.allow_low_precision(reason="bf16 O"):
                        nc.gpsimd.tensor_mul(ofin[:], o_sb[0:64, :],
                                             rb_sb[:])
                    for h in range(HL):
                        for half in range(2):
                            s8 = 2 * j + half
                            nc.sync.dma_start(
                                inb[b].ap()[s8, 64 * h:64 * (h + 1), :],
                                ofin[:, 512 * h + 256 * half:
                                     512 * h + 256 * (half + 1)])

            def emit_a2a(b):
                if sim:
                    # stand-in with comparable cost for the cost-model sim
                    nc.sync.dma_start(outb[b].ap(), inb[b].ap())
                else:
                    nc.gpsimd.collective_compute(
                        "AllToAll", mybir.AluOpType.bypass,
                        replica_groups=[list(range(NCORES))],
                        ins=[inb[b].ap().opt()], outs=[outb[b].ap().opt()],
                    )

            def emit_outproj(b):
                ots = []
                for s8 in range(NCORES):
                    ot = otpool.tile([128, PIECE], BF16, tag="ot")
                    nc.sync.dma_start(ot[:], outb[b].ap()[s8])
                    ots.append(ot)
                for mcol in range(NKC):
                    pp = sps.tile([128, 1024], F32, tag="s_ps", name="pp")
                    for s8 in range(NCORES):
                        nc.tensor.matmul(
                            pp[:, 0:PIECE],
                            wp_sb[:, s8, 128 * mcol:128 * (mcol + 1)],
                            ots[s8][:], start=(s8 == 0),
                            stop=(s8 == NCORES - 1))
                    osb = projpool.tile([128, PIECE], BF16, tag="osb")
                    with nc.allow_low_precision(reason="bf16 out"):
                        nc.vector.tensor_scalar_add(osb[:], pp[:, 0:PIECE],
                                                    bp_sb[:, mcol:mcol + 1])
                    nc.sync.dma_start(
                        outp[128 * mcol:128 * (mcol + 1),
                             PIECE * b:PIECE * (b + 1)], osb[:])

            emit_qkv(0)
            for b in range(B):
                emit_attn(b)
                emit_a2a(b)
                if b + 1 < B:
                    emit_qkv(b + 1)
                emit_outproj(b)
    nc.compile()
    return nc


def _get_nc():
    if "nc" not in _CACHE:
        _CACHE["nc"] = _build()
    return _CACHE["nc"]


def kernel(x, W_qkv, b_qkv, W_proj, b_proj):
    x = np.asarray(x, dtype=np.float32)
    W_qkv = np.asarray(W_qkv, dtype=np.float32)
    b_qkv = np.asarray(b_qkv, dtype=np.float32)
    W_proj = np.asarray(W_proj, dtype=np.float32)
    b_proj = np.asarray(b_proj, dtype=np.float32)

    BF = ml_dtypes.bfloat16
    scale = 1.0 / np.sqrt(HD)
    xt = np.ascontiguousarray(x.reshape(B * T, C).T).astype(BF)  # [C, B*T]
    wp = np.ascontiguousarray(W_proj).astype(BF)                 # [C, C]
    bp = np.ascontiguousarray(b_proj.reshape(NKC, 128).T)        # [128, 8]

    qw = W_qkv[:, 0:C]
    kw = W_qkv[:, C:2 * C]
    vw = W_qkv[:, 2 * C:3 * C]
    qb, kb, vb = b_qkv[0:C], b_qkv[C:2 * C], b_qkv[2 * C:3 * C]

    in_maps = []
    for c in range(NCORES):
        cols = slice(2 * c * HD, (2 * c + 2) * HD)  # this core's 128 dims
        bq = np.stack([qb[cols] * scale, kb[cols], vb[cols]], axis=1)  # [128,3]
        in_maps.append({
            "xt": xt,
            "wq": np.ascontiguousarray(qw[:, cols] * scale).astype(BF),
            "wk": np.ascontiguousarray(kw[:, cols]).astype(BF),
            "wv": np.ascontiguousarray(vw[:, cols]).astype(BF),
            "wp": wp,
            "bqkv": np.ascontiguousarray(bq),
            "bp": bp,
        })

    nc = _get_nc()
    _CACHE["last_in_maps"] = in_maps
    res = run_bass_kernel_spmd(nc, in_maps, core_ids=list(range(NCORES)))

    # outp[c]: [C, B*PIECE] (cols: b-major, then 256 tokens of piece c)
    allo = np.stack([np.asarray(res.results[c]["outp"], dtype=np.float32)
                     for c in range(NCORES)])
    allo = allo.reshape(NCORES, C, B, PIECE)       # [c, ch, b, u]
    out = allo.transpose(2, 0, 3, 1).reshape(B, T, C)
    return np.ascontiguousarray(out)


# revision 7
# speedup vs baseline: 1.2102x; 1.1733x over previous
"""Causal self-attention (B=4, T=2048, C=1024, H=16) on 8 trn2 NeuronCores.

Sharding: head-pair parallel. Core c owns heads {2c, 2c+1} for all 4 batches.
 - host: x pre-transposed to xT [C, B*T] bf16; W_qkv pre-sliced per core into
   wq/wk/wv [C, 128] bf16 (softmax scale folded into wq), W_proj bf16,
   f32 biases, and a packed bf16 constants tensor (identity, causal tril
   mask, ones) so the kernel needs no GpSimd ucode at all.
 - device per core: qkv projections as bf16 matmuls (x staged as one
   [128, 2048] tile per contraction chunk per batch, prefetched a full batch
   ahead); vT is PE-transposed per 128-tile (both heads in one [128,128]
   bf16 transpose) into per-head v [T, 64] tiles + ones column (vh).
 - attention in S^T orientation: S^T[tk, tq] = kT.T@qT tiles [128, 512],
   diagonal tiles first. P^T = exp(S^T) on ScalarE (PSUM -> SBUF bf16); the
   128x128 causal wedge of diagonal tiles is zeroed on DVE by multiplying
   with a host-shipped 0/1 tril mask (exp of unmasked future logits is
   finite so this is safe). O-matmul lhsT = [v_h | ones] (M=65) gives O^T
   and the softmax denominator in one pass. Normalize = PSUM->SBUF copy at
   chunk end; the reciprocal + ones-broadcast matmul + multiply + stores are
   deferred into the next chunk's loop so they never head-of-line block PE.
 - per-batch AllToAll (512 KB/rank bf16) reshards O^T head->token shards;
   qkv(b+1) is issued between a2a(b) and outproj(b) so PE never waits on the
   collective. Column-parallel out-projection, bf16 out^T, host upcasts.
"""
import numpy as np
import ml_dtypes
import concourse.bacc as bacc
import concourse.mybir as mybir
import concourse.tile as tile
from concourse.bass_utils import run_bass_kernel_spmd

F32 = mybir.dt.float32
BF16 = mybir.dt.bfloat16
Exp = mybir.ActivationFunctionType.Exp

NCORES = 8
B, T, C, H = 4, 2048, 1024, 16
HD = C // H          # 64
HL = H // NCORES     # 2 heads per core
D2 = HL * HD         # 128 rows of local head-pair dims
TB = T               # tokens per batch
NKC = C // 128       # 8 contraction chunks
NCH = TB // 512      # 4 tq chunks per batch
NTK = TB // 128      # 16 tk tiles per batch
PIECE = TB // NCORES  # 256 tokens per (batch, core) piece after AllToAll

# packed constants layout (bf16, [128, 480]):
#   cols 0:128  identity; 128:384 tril mask x2 heads; 384:416 ones (vh);
#   row 0 cols 416:480 ones (rb lhsT)
CN_ID = 0
CN_MASK = 128
CN_ONES = 384
CN_ONESR = 416
CN_W = 480

_CACHE = {}


def _build(sim=False):
    nc = bacc.Bacc("TRN2", target_bir_lowering=False, debug=False,
                   num_devices=1 if sim else NCORES)
    xt = nc.dram_tensor("xt", [C, B * T], BF16, kind="ExternalInput").ap()
    wq = nc.dram_tensor("wq", [C, D2], BF16, kind="ExternalInput").ap()
    wk = nc.dram_tensor("wk", [C, D2], BF16, kind="ExternalInput").ap()
    wv = nc.dram_tensor("wv", [C, D2], BF16, kind="ExternalInput").ap()
    wp = nc.dram_tensor("wp", [C, C], BF16, kind="ExternalInput").ap()
    cn = nc.dram_tensor("cn", [128, CN_W], BF16, kind="ExternalInput").ap()
    bqkv = nc.dram_tensor("bqkv", [D2, 3], F32, kind="ExternalInput").ap()
    bp = nc.dram_tensor("bp", [128, NKC], F32, kind="ExternalInput").ap()
    outp = nc.dram_tensor("outp", [C, B * PIECE], BF16,
                          kind="ExternalOutput").ap()

    inb = [nc.dram_tensor(f"inb{b}", [NCORES, D2, PIECE], BF16)
           for b in range(B)]
    outb = [nc.dram_tensor(f"outb{b}", [NCORES, D2, PIECE], BF16)
            for b in range(B)]

    with tile.TileContext(nc) as tc:
        with (
            tc.tile_pool(name="const", bufs=1) as cpool,
            tc.tile_pool(name="w", bufs=1) as wpool,
            tc.tile_pool(name="xt", bufs=16) as xpool,
            tc.tile_pool(name="qk", bufs=2) as qkpool,
            tc.tile_pool(name="vstg", bufs=1) as vstgpool,
            tc.tile_pool(name="vh", bufs=2) as vhpool,
            tc.tile_pool(name="pt", bufs=5) as ptpool,
            tc.tile_pool(name="small", bufs=3) as smallpool,
            tc.tile_pool(name="ofin", bufs=3) as ofinpool,
            tc.tile_pool(name="proj", bufs=3) as projpool,
            tc.tile_pool(name="otp", bufs=2) as otpool,
            tc.tile_pool(name="mm", bufs=2, space="PSUM") as mmps,
            tc.tile_pool(name="s", bufs=2, space="PSUM") as sps,
            tc.tile_pool(name="o", bufs=1, space="PSUM") as ops,
        ):
            # ---- constants / weights ----
            cn_sb = cpool.tile([128, CN_W], BF16)
            nc.sync.dma_start(cn_sb[:], cn)
            identb = cn_sb[:, CN_ID:CN_ID + 128]
            maskb = cn_sb[:, CN_MASK:CN_MASK + 256].rearrange(
                "p (g c) -> p g c", g=HL)
            onesb = cn_sb[:, CN_ONES:CN_ONES + 32]
            onesr = cn_sb[0:1, CN_ONESR:CN_ONESR + 64]

            wq_sb = wpool.tile([128, NKC, D2], BF16)
            wk_sb = wpool.tile([128, NKC, D2], BF16)
            wv_sb = wpool.tile([128, NKC, D2], BF16)
            for t, d in ((wq_sb, wq), (wk_sb, wk), (wv_sb, wv)):
                nc.sync.dma_start(
                    t[:], d.rearrange("(kc p) m -> p kc m", p=128))
            wp_sb = wpool.tile([128, NKC, C], BF16)
            bqkv_sb = cpool.tile([D2, 3], F32)
            nc.sync.dma_start(bqkv_sb[:], bqkv)
            bp_sb = cpool.tile([128, NKC], F32)
            nc.sync.dma_start(bp_sb[:], bp)

            qT = {}
            kT = {}
            vh = {}
            xts = {}

            def emit_xt_loads(b):
                xts[b] = []
                for kc in range(NKC):
                    xtile = xpool.tile([128, TB], BF16, name="xtile")
                    nc.sync.dma_start(
                        xtile[:],
                        xt[128 * kc:128 * (kc + 1), b * TB:(b + 1) * TB])
                    xts[b].append(xtile)

            def emit_qkv(b):
                qT[b] = qkpool.tile([D2, TB], BF16, tag="qT", name="qT")
                kT[b] = qkpool.tile([D2, TB], BF16, tag="kT", name="kT")
                vT = vstgpool.tile([D2, TB], BF16)
                for n in range(NCH):
                    if b == 0 and n < 2:
                        # W_proj loads behind batch-0 x loads
                        for g in (2 * n, 2 * n + 1):
                            nc.sync.dma_start(
                                wp_sb[:, 2 * g:2 * (g + 1), :],
                                wp[256 * g:256 * (g + 1), :].rearrange(
                                    "(two p) m -> p two m", p=128))
                    for w_sb, col, dst in ((wq_sb, 0, qT[b]),
                                           (wk_sb, 1, kT[b]),
                                           (wv_sb, 2, vT)):
                        ps = sps.tile([128, 1024], F32, tag="s_ps",
                                      name="ps")
                        for kc in range(NKC):
                            nc.tensor.matmul(
                                ps[:, 0:512], w_sb[:, kc, :],
                                xts[b][kc][:, 512 * n:512 * (n + 1)],
                                start=(kc == 0), stop=(kc == NKC - 1))
                        with nc.allow_low_precision(reason="bf16 qkv"):
                            nc.vector.tensor_scalar_add(
                                dst[:, 512 * n:512 * (n + 1)], ps[:, 0:512],
                                bqkv_sb[:, col:col + 1])
                del xts[b]

                # ---- v transpose: vT [d2, T] -> vh [tk, d] per head + ones
                # layout: vh[:, h*(NTK*65) + tk*65 + (0:64)] = v_h tile tk
                vh[b] = vhpool.tile([128, HL * NTK * 65], BF16, tag="vh",
                                    name="vh")
                with nc.allow_low_precision(reason="bf16 v ones"):
                    nc.vector.tensor_copy(vh[b][:, 64::65], onesb[:])
                for tk0 in range(0, NTK, 4):
                    vt_ps = mmps.tile([128, 512], BF16, tag="vtps")
                    for i in range(4):
                        nc.tensor.transpose(
                            vt_ps[:, 128 * i:128 * (i + 1)],
                            vT[:, 128 * (tk0 + i):128 * (tk0 + i + 1)],
                            identb)
                    with nc.allow_low_precision(reason="bf16 v"):
                        nc.vector.tensor_copy(
                            vh[b][:].rearrange(
                                "p (h tk c) -> p h tk c",
                                h=HL, tk=NTK)[:, :, tk0:tk0 + 4, 0:64],
                            vt_ps[:].rearrange(
                                "p (i h c) -> p h i c", i=4, h=HL))

            pending = []

            def flush_pending():
                while pending:
                    pending.pop(0)()

            def emit_attn(b):
                if b + 1 < B:
                    emit_xt_loads(b + 1)
                for j in range(NCH):
                    o_ps = ops.tile([65, 1024], F32, tag="o")
                    # diagonal tiles first: m=0 is full width and opens the
                    # O accumulation; the wedge masks pipeline behind exp
                    seq = list(range(4 * j, 4 * j + 4)) + list(range(4 * j))
                    for idx, tk in enumerate(seq):
                        m = tk - 4 * j
                        z = 128 * m if m >= 0 else 0
                        s_ps = sps.tile([128, 1024], F32, tag="s_ps",
                                        name="s_ps")
                        for h in range(HL):
                            nc.tensor.matmul(
                                s_ps[:, 512 * h + z:512 * (h + 1)],
                                kT[b][64 * h:64 * (h + 1),
                                      128 * tk:128 * (tk + 1)],
                                qT[b][64 * h:64 * (h + 1),
                                      512 * j + z:512 * (j + 1)],
                                start=True, stop=True)
                        pt = ptpool.tile([128, 1024], BF16, tag="pt")
                        if z:
                            exp_src = s_ps[:].rearrange(
                                "p (g c) -> p g c", g=2)[:, :, z:]
                            exp_dst = pt[:].rearrange(
                                "p (g c) -> p g c", g=2)[:, :, z:]
                            nc.scalar.activation(exp_dst, exp_src, Exp)
                        else:
                            nc.scalar.activation(pt[:], s_ps[:], Exp)
                        if m >= 0:
                            # zero the causal wedge: keep where tq_l >= tk_l
                            ptv = pt[:].rearrange(
                                "p (g c) -> p g c", g=2)[:, :, z:z + 128]
                            with nc.allow_low_precision(reason="bf16 mask"):
                                nc.vector.tensor_mul(ptv, ptv, maskb)
                        for h in range(HL):
                            nc.tensor.matmul(
                                o_ps[0:65, 512 * h + z:512 * (h + 1)],
                                vh[b][:, (h * NTK + tk) * 65:
                                      (h * NTK + tk + 1) * 65],
                                pt[:, 512 * h + z:512 * (h + 1)],
                                start=(idx == 0), stop=(idx == len(seq) - 1))
                        if idx == 2:
                            flush_pending()

                    # ---- normalize: only the PSUM->SBUF copy now; the rest
                    # is deferred so it never head-of-line blocks PE
                    o_sb = smallpool.tile([65, 1024], BF16, tag="osb2",
                                          name="o_sb")
                    with nc.allow_low_precision(reason="bf16 O"):
                        nc.vector.tensor_copy(o_sb[:], o_ps[:])

                    def norm_tail(b=b, j=j, o_sb=o_sb):
                        r_sb = smallpool.tile([1, 1024], BF16, tag="r",
                                              name="r_sb")
                        with nc.allow_low_precision(reason="softmax denom"):
                            nc.vector.reciprocal(r_sb[:], o_sb[64:65, :])
                        rb_ps = sps.tile([128, 1024], F32, tag="s_ps",
                                         name="rb_ps")
                        nc.tensor.matmul(rb_ps[0:64, :], onesr, r_sb[:],
                                         start=True, stop=True)
                        rb_sb = smallpool.tile([64, 1024], BF16, tag="rb",
                                               name="rb_sb")
                        with nc.allow_low_precision(reason="bf16 rb"):
                            nc.vector.tensor_copy(rb_sb[:], rb_ps[0:64, :])
                        ofin = ofinpool.tile([64, 1024], BF16, name="ofin")
                        with nc.allow_low_precision(reason="bf16 O"):
                            nc.vector.tensor_mul(ofin[:], o_sb[0:64, :],
                                                 rb_sb[:])
                        for h in range(HL):
                            nc.sync.dma_start(
                                inb[b].ap()[2 * j:2 * j + 2,
                                            64 * h:64 * (h + 1), :]
                                .rearrange("s p u -> p s u"),
                                ofin[:, 512 * h:512 * (h + 1)].rearrange(
                                    "p (s u) -> p s u", s=2))

                    pending.append(norm_tail)
                flush_pending()

            def emit_a2a(b):
                if sim:
                    # stand-in with comparable cost for the cost-model sim
                    nc.sync.dma_start(outb[b].ap(), inb[b].ap())
                else:
                    nc.gpsimd.collective_compute(
                        "AllToAll", mybir.AluOpType.bypass,
                        replica_groups=[list(range(NCORES))],
                        ins=[inb[b].ap().opt()], outs=[outb[b].ap().opt()],
                    )

            def emit_outproj(b):
                ot = otpool.tile([128, NCORES, PIECE], BF16, name="ot")
                nc.sync.dma_start(
                    ot[:], outb[b].ap().rearrange("s p u -> p s u"))
                for mc2 in range(NKC // 2):
                    osb = projpool.tile([128, 2, PIECE], BF16, tag="osb",
                                        name="osb")
                    for sub in range(2):
                        mcol = 2 * mc2 + sub
                        pp = sps.tile([128, 1024], F32, tag="s_ps",
                                      name="pp")
                        for s8 in range(NCORES):
                            nc.tensor.matmul(
                                pp[:, 0:PIECE],
                                wp_sb[:, s8, 128 * mcol:128 * (mcol + 1)],
                                ot[:, s8, :], start=(s8 == 0),
                                stop=(s8 == NCORES - 1))
                        with nc.allow_low_precision(reason="bf16 out"):
                            nc.vector.tensor_scalar_add(
                                osb[:, sub, :], pp[:, 0:PIECE],
                                bp_sb[:, mcol:mcol + 1])
                    nc.sync.dma_start(
                        outp[256 * mc2:256 * (mc2 + 1),
                             PIECE * b:PIECE * (b + 1)].rearrange(
                                 "(two p) u -> p two u", p=128),
                        osb[:])

            emit_xt_loads(0)
            emit_qkv(0)
            for b in range(B):
                emit_attn(b)
                emit_a2a(b)
                if b + 1 < B:
                    emit_qkv(b + 1)
                emit_outproj(b)
    nc.compile()
    return nc


def _get_nc():
    if "nc" not in _CACHE:
        _CACHE["nc"] = _build()
    return _CACHE["nc"]


def kernel(x, W_qkv, b_qkv, W_proj, b_proj):
    x = np.asarray(x, dtype=np.float32)
    W_qkv = np.asarray(W_qkv, dtype=np.float32)
    b_qkv = np.asarray(b_qkv, dtype=np.float32)
    W_proj = np.asarray(W_proj, dtype=np.float32)
    b_proj = np.asarray(b_proj, dtype=np.float32)

    BF = ml_dtypes.bfloat16
    scale = 1.0 / np.sqrt(HD)
    xt = np.ascontiguousarray(x.reshape(B * T, C).T).astype(BF)  # [C, B*T]
    wp = np.ascontiguousarray(W_proj).astype(BF)                 # [C, C]
    bp = np.ascontiguousarray(b_proj.reshape(NKC, 128).T)        # [128, 8]

    cn = np.zeros((128, CN_W), dtype=BF)
    cn[:, CN_ID:CN_ID + 128] = np.eye(128, dtype=np.float32)
    tril = (np.arange(128)[None, :] >= np.arange(128)[:, None])
    cn[:, CN_MASK:CN_MASK + 128] = tril.astype(np.float32)
    cn[:, CN_MASK + 128:CN_MASK + 256] = tril.astype(np.float32)
    cn[:, CN_ONES:CN_ONES + 32] = 1.0
    cn[0, CN_ONESR:CN_ONESR + 64] = 1.0

    qw = W_qkv[:, 0:C]
    kw = W_qkv[:, C:2 * C]
    vw = W_qkv[:, 2 * C:3 * C]
    qb, kb, vb = b_qkv[0:C], b_qkv[C:2 * C], b_qkv[2 * C:3 * C]

    in_maps = []
    for c in range(NCORES):
        cols = slice(2 * c * HD, (2 * c + 2) * HD)  # this core's 128 dims
        bq = np.stack([qb[cols] * scale, kb[cols], vb[cols]], axis=1)  # [128,3]
        in_maps.append({
            "xt": xt,
            "wq": np.ascontiguousarray(qw[:, cols] * scale).astype(BF),
            "wk": np.ascontiguousarray(kw[:, cols]).astype(BF),
            "wv": np.ascontiguousarray(vw[:, cols]).astype(BF),
            "wp": wp,
            "cn": cn,
            "bqkv": np.ascontiguousarray(bq),
            "bp": bp,
        })

    nc = _get_nc()
    _CACHE["last_in_maps"] = in_maps
    res = run_bass_kernel_spmd(nc, in_maps, core_ids=list(range(NCORES)))

    # outp[c]: [C, B*PIECE] (cols: b-major, then 256 tokens of piece c)
    allo = np.stack([np.asarray(res.results[c]["outp"], dtype=np.float32)
                     for c in range(NCORES)])
    allo = allo.reshape(NCORES, C, B, PIECE)       # [c, ch, b, u]
    out = allo.transpose(2, 0, 3, 1).reshape(B, T, C)
    return np.ascontiguousarray(out)


# revision 8
# speedup vs baseline: 1.2855x; 1.0622x over previous
"""Causal self-attention (B=4, T=2048, C=1024, H=16) on 8 trn2 NeuronCores.

Sharding: head-pair parallel. Core c owns heads {2c, 2c+1} for all 4 batches.
 - host: x pre-transposed to xT [C, B*T] bf16; W_qkv pre-sliced per core into
   wq/wk/wv [C, 128] bf16 (softmax scale folded into wq), W_proj bf16,
   f32 biases, and a packed bf16 constants tensor (identity, causal tril
   mask, ones) so the kernel needs no GpSimd compute ucode.
 - software pipeline: the attention tile loop of batch b doubles as the
   scheduler for everything else. qkv projections of batch b+1, the V
   transposes of b+1, and the out-projection of b-1 are emitted as small
   self-contained "filler units" between attention tiles, so the PE stays
   busy instead of pacing on ScalarE's exp. x tiles are prefetched two
   batches ahead ([128, 2048] per contraction chunk, one DMA each).
 - attention in S^T orientation: S^T[tk, tq] = kT.T@qT bf16 tiles
   [128, 512], diagonal tiles first. P^T = exp(S^T) on ScalarE (PSUM->SBUF
   bf16); the 128x128 causal wedge of diagonal tiles is zeroed on DVE by
   multiplying with a host-shipped 0/1 tril mask (exp of unmasked future
   logits is finite so this is safe). O-matmul lhsT = [v_h | ones] (M=65)
   gives O^T and the softmax denominator in one pass. Normalize: only the
   PSUM->SBUF copy happens at chunk end; reciprocal + ones-broadcast matmul
   + multiply + stores are deferred into the next chunk's tile loop.
 - per-batch AllToAll (512 KB/rank bf16) reshards O^T head->token shards.
   SBUF->DRAM stores go through the (otherwise idle) GpSimd DGE queue so
   they never head-of-line block the sync DGE queue that feeds loads and
   the collective. Column-parallel out-projection, bf16 out^T, host
   upcasts and reassembles.
"""
import numpy as np
import ml_dtypes
import concourse.bacc as bacc
import concourse.mybir as mybir
import concourse.tile as tile
from concourse.bass_utils import run_bass_kernel_spmd

F32 = mybir.dt.float32
BF16 = mybir.dt.bfloat16
Exp = mybir.ActivationFunctionType.Exp

NCORES = 8
B, T, C, H = 4, 2048, 1024, 16
HD = C // H          # 64
HL = H // NCORES     # 2 heads per core
D2 = HL * HD         # 128 rows of local head-pair dims
TB = T               # tokens per batch
NKC = C // 128       # 8 contraction chunks
NCH = TB // 512      # 4 tq chunks per batch
NTK = TB // 128      # 16 tk tiles per batch
PIECE = TB // NCORES  # 256 tokens per (batch, core) piece after AllToAll

# packed constants layout (bf16, [128, 480]):
#   cols 0:128  identity; 128:384 tril mask x2 heads; 384:416 ones (vh);
#   row 0 cols 416:480 ones (rb lhsT)
CN_ID = 0
CN_MASK = 128
CN_ONES = 384
CN_ONESR = 416
CN_W = 480

_CACHE = {}


def _build(sim=False):
    nc = bacc.Bacc("TRN2", target_bir_lowering=False, debug=False,
                   num_devices=1 if sim else NCORES)
    xt = nc.dram_tensor("xt", [C, B * T], BF16, kind="ExternalInput").ap()
    wq = nc.dram_tensor("wq", [C, D2], BF16, kind="ExternalInput").ap()
    wk = nc.dram_tensor("wk", [C, D2], BF16, kind="ExternalInput").ap()
    wv = nc.dram_tensor("wv", [C, D2], BF16, kind="ExternalInput").ap()
    wp = nc.dram_tensor("wp", [C, C], BF16, kind="ExternalInput").ap()
    cn = nc.dram_tensor("cn", [128, CN_W], BF16, kind="ExternalInput").ap()
    bqkv = nc.dram_tensor("bqkv", [D2, 3], F32, kind="ExternalInput").ap()
    bp = nc.dram_tensor("bp", [128, NKC], F32, kind="ExternalInput").ap()
    outp = nc.dram_tensor("outp", [C, B * PIECE], BF16,
                          kind="ExternalOutput").ap()

    inb = [nc.dram_tensor(f"inb{b}", [NCORES, D2, PIECE], BF16)
           for b in range(B)]
    outb = [nc.dram_tensor(f"outb{b}", [NCORES, D2, PIECE], BF16)
            for b in range(B)]

    with tile.TileContext(nc) as tc:
        with (
            tc.tile_pool(name="const", bufs=1) as cpool,
            tc.tile_pool(name="w", bufs=1) as wpool,
            tc.tile_pool(name="xt", bufs=16) as xpool,
            tc.tile_pool(name="qk", bufs=2) as qkpool,
            tc.tile_pool(name="vstg", bufs=1) as vstgpool,
            tc.tile_pool(name="vh", bufs=2) as vhpool,
            tc.tile_pool(name="pt", bufs=5) as ptpool,
            tc.tile_pool(name="small", bufs=3) as smallpool,
            tc.tile_pool(name="ofin", bufs=3) as ofinpool,
            tc.tile_pool(name="proj", bufs=3) as projpool,
            tc.tile_pool(name="otp", bufs=2) as otpool,
            tc.tile_pool(name="mm", bufs=1, space="PSUM") as mmps,
            tc.tile_pool(name="s", bufs=2, space="PSUM") as sps,
            tc.tile_pool(name="o", bufs=1, space="PSUM") as ops,
        ):
            # ---- constants / weights, ordered so batch-0 qkv starts asap
            wq_sb = wpool.tile([128, NKC, D2], BF16)
            wk_sb = wpool.tile([128, NKC, D2], BF16)
            wv_sb = wpool.tile([128, NKC, D2], BF16)
            nc.sync.dma_start(
                wq_sb[:], wq.rearrange("(kc p) m -> p kc m", p=128))

            xts = {}

            def emit_xt_loads(b, striped=False):
                xts[b] = [xpool.tile([128, TB], BF16, name="xtile")
                          for _ in range(NKC)]
                if striped:
                    # chunk-major stripes so batch-0 compute starts early
                    for n in range(NCH):
                        for kc in range(NKC):
                            nc.sync.dma_start(
                                xts[b][kc][:, 512 * n:512 * (n + 1)],
                                xt[128 * kc:128 * (kc + 1),
                                   b * TB + 512 * n:b * TB + 512 * (n + 1)])
                else:
                    for kc in range(NKC):
                        nc.sync.dma_start(
                            xts[b][kc][:],
                            xt[128 * kc:128 * (kc + 1), b * TB:(b + 1) * TB])

            emit_xt_loads(0, striped=True)
            nc.sync.dma_start(
                wk_sb[:], wk.rearrange("(kc p) m -> p kc m", p=128))
            bqkv_sb = cpool.tile([D2, 3], F32)
            nc.sync.dma_start(bqkv_sb[:], bqkv)
            nc.sync.dma_start(
                wv_sb[:], wv.rearrange("(kc p) m -> p kc m", p=128))
            cn_sb = cpool.tile([128, CN_W], BF16)
            nc.sync.dma_start(cn_sb[:], cn)
            identb = cn_sb[:, CN_ID:CN_ID + 128]
            maskb = cn_sb[:, CN_MASK:CN_MASK + 256].rearrange(
                "p (g c) -> p g c", g=HL)
            onesb = cn_sb[:, CN_ONES:CN_ONES + 32]
            onesr = cn_sb[0:1, CN_ONESR:CN_ONESR + 64]

            emit_xt_loads(1)
            wp_sb = wpool.tile([128, NKC, C], BF16)
            for g in range(4):
                nc.sync.dma_start(
                    wp_sb[:, 2 * g:2 * (g + 1), :],
                    wp[256 * g:256 * (g + 1), :].rearrange(
                        "(two p) m -> p two m", p=128))
            bp_sb = cpool.tile([128, NKC], F32)
            nc.sync.dma_start(bp_sb[:], bp)

            qT = {}
            kT = {}
            vh = {}
            vT = {}

            def qkv_units(b):
                """12 qkv matmul groups + 4 v-transpose groups for batch b."""
                qT[b] = qkpool.tile([D2, TB], BF16, tag="qT", name="qT")
                kT[b] = qkpool.tile([D2, TB], BF16, tag="kT", name="kT")
                vT[b] = vstgpool.tile([D2, TB], BF16, name="vT")
                units = []

                def qkv_group(n, col, w_sb, dst, b=b):
                    def emit():
                        ps = mmps.tile([128, 512], F32, tag="qps",
                                       name="ps")
                        for kc in range(NKC):
                            nc.tensor.matmul(
                                ps[:], w_sb[:, kc, :],
                                xts[b][kc][:, 512 * n:512 * (n + 1)],
                                start=(kc == 0), stop=(kc == NKC - 1))
                        with nc.allow_low_precision(reason="bf16 qkv"):
                            nc.vector.tensor_scalar_add(
                                dst[:, 512 * n:512 * (n + 1)], ps[:],
                                bqkv_sb[:, col:col + 1])
                        if n == NCH - 1 and col == 2:
                            del xts[b]
                    return emit

                for n in range(NCH):
                    for col, (w_sb, dst) in enumerate(
                            ((wq_sb, qT[b]), (wk_sb, kT[b]),
                             (wv_sb, vT[b]))):
                        units.append(qkv_group(n, col, w_sb, dst))

                def vt_group(tk0, b=b):
                    def emit():
                        # layout: vh[:, h*(NTK*65) + tk*65 + (0:64)] = v tile
                        if tk0 == 0:
                            vh[b] = vhpool.tile([128, HL * NTK * 65], BF16,
                                                tag="vh", name="vh")
                            with nc.allow_low_precision(reason="ones"):
                                nc.vector.tensor_copy(vh[b][:, 64::65],
                                                      onesb[:])
                        vt_ps = mmps.tile([128, 512], BF16, tag="vtps")
                        for i in range(4):
                            nc.tensor.transpose(
                                vt_ps[:, 128 * i:128 * (i + 1)],
                                vT[b][:, 128 * (tk0 + i):
                                      128 * (tk0 + i + 1)],
                                identb)
                        with nc.allow_low_precision(reason="bf16 v"):
                            nc.vector.tensor_copy(
                                vh[b][:].rearrange(
                                    "p (h tk c) -> p h tk c",
                                    h=HL, tk=NTK)[:, :, tk0:tk0 + 4, 0:64],
                                vt_ps[:].rearrange(
                                    "p (i h c) -> p h i c", i=4, h=HL))
                    return emit

                for tk0 in range(0, NTK, 4):
                    units.append(vt_group(tk0))
                return units

            ot = {}

            def emit_ot_load(b):
                ot[b] = otpool.tile([128, NCORES, PIECE], BF16, name="ot")
                nc.sync.dma_start(
                    ot[b][:], outb[b].ap().rearrange("s p u -> p s u"))

            def outproj_units(b):
                """4 out-projection column-pair groups for batch b."""
                units = []

                def op_group(mc2, b=b):
                    def emit():
                        osb = projpool.tile([128, 2, PIECE], BF16,
                                            tag="osb", name="osb")
                        for sub in range(2):
                            mcol = 2 * mc2 + sub
                            pp = mmps.tile([128, 512], F32, tag="qps",
                                           name="pp")
                            for s8 in range(NCORES):
                                nc.tensor.matmul(
                                    pp[:, 0:PIECE],
                                    wp_sb[:, s8,
                                          128 * mcol:128 * (mcol + 1)],
                                    ot[b][:, s8, :], start=(s8 == 0),
                                    stop=(s8 == NCORES - 1))
                            with nc.allow_low_precision(reason="bf16 out"):
                                nc.vector.tensor_scalar_add(
                                    osb[:, sub, :], pp[:, 0:PIECE],
                                    bp_sb[:, mcol:mcol + 1])
                        nc.gpsimd.dma_start(
                            outp[256 * mc2:256 * (mc2 + 1),
                                 PIECE * b:PIECE * (b + 1)].rearrange(
                                     "(two p) u -> p two u", p=128),
                            osb[:])
                    return emit

                for mc2 in range(NKC // 2):
                    units.append(op_group(mc2))
                return units

            def interleave(a, bl):
                """a with elements of bl spread through (a leads)."""
                if not bl:
                    return list(a)
                out = []
                step = max(1, len(a) // (len(bl) + 1))
                bi = 0
                for i, u in enumerate(a):
                    out.append(u)
                    if bi < len(bl) and (i + 1) % step == 0 and i >= 2:
                        out.append(bl[bi])
                        bi += 1
                out.extend(bl[bi:])
                return out

            pending = []

            def flush_pending():
                while pending:
                    pending.pop(0)()

            def emit_attn(b, units):
                if b + 2 < B:
                    emit_xt_loads(b + 2)
                n_tiles = sum(4 * j + 4 for j in range(NCH))
                done = 0
                emitted = 0
                for j in range(NCH):
                    o_ps = ops.tile([65, 1024], F32, tag="o")
                    # diagonal tiles first: m=0 is full width and opens the
                    # O accumulation; wedge masks pipeline behind exp
                    seq = list(range(4 * j, 4 * j + 4)) + list(range(4 * j))
                    for idx, tk in enumerate(seq):
                        m = tk - 4 * j
                        z = 128 * m if m >= 0 else 0
                        s_ps = sps.tile([128, 1024], F32, tag="s_ps",
                                        name="s_ps")
                        for h in range(HL):
                            nc.tensor.matmul(
                                s_ps[:, 512 * h + z:512 * (h + 1)],
                                kT[b][64 * h:64 * (h + 1),
                                      128 * tk:128 * (tk + 1)],
                                qT[b][64 * h:64 * (h + 1),
                                      512 * j + z:512 * (j + 1)],
                                start=True, stop=True)
                        pt = ptpool.tile([128, 1024], BF16, tag="pt")
                        if z:
                            exp_src = s_ps[:].rearrange(
                                "p (g c) -> p g c", g=2)[:, :, z:]
                            exp_dst = pt[:].rearrange(
                                "p (g c) -> p g c", g=2)[:, :, z:]
                            nc.scalar.activation(exp_dst, exp_src, Exp)
                        else:
                            nc.scalar.activation(pt[:], s_ps[:], Exp)
                        if m >= 0:
                            # zero the causal wedge: keep where tq_l >= tk_l
                            ptv = pt[:].rearrange(
                                "p (g c) -> p g c", g=2)[:, :, z:z + 128]
                            with nc.allow_low_precision(reason="mask"):
                                nc.vector.tensor_mul(ptv, ptv, maskb)
                        for h in range(HL):
                            nc.tensor.matmul(
                                o_ps[0:65, 512 * h + z:512 * (h + 1)],
                                vh[b][:, (h * NTK + tk) * 65:
                                      (h * NTK + tk + 1) * 65],
                                pt[:, 512 * h + z:512 * (h + 1)],
                                start=(idx == 0), stop=(idx == len(seq) - 1))
                        done += 1
                        if idx == 2:
                            flush_pending()
                        # pace filler so it finishes ~4 tiles before the end
                        target = min(len(units),
                                     done * len(units) // max(1, n_tiles - 4))
                        while emitted < target:
                            units[emitted]()
                            emitted += 1

                    # normalize: only the PSUM->SBUF copy now; the rest is
                    # deferred so it never head-of-line blocks PE
                    o_sb = smallpool.tile([65, 1024], BF16, tag="osb2",
                                          name="o_sb")
                    with nc.allow_low_precision(reason="bf16 O"):
                        nc.vector.tensor_copy(o_sb[:], o_ps[:])

                    def norm_tail(b=b, j=j, o_sb=o_sb):
                        r_sb = smallpool.tile([1, 1024], BF16, tag="r",
                                              name="r_sb")
                        with nc.allow_low_precision(reason="denom"):
                            nc.vector.reciprocal(r_sb[:], o_sb[64:65, :])
                        rb_ps = sps.tile([128, 1024], F32, tag="s_ps",
                                         name="rb_ps")
                        nc.tensor.matmul(rb_ps[0:64, :], onesr, r_sb[:],
                                         start=True, stop=True)
                        rb_sb = smallpool.tile([64, 1024], BF16, tag="rb",
                                               name="rb_sb")
                        with nc.allow_low_precision(reason="bf16 rb"):
                            nc.vector.tensor_copy(rb_sb[:], rb_ps[0:64, :])
                        ofin = ofinpool.tile([64, 1024], BF16, name="ofin")
                        with nc.allow_low_precision(reason="bf16 O"):
                            nc.vector.tensor_mul(ofin[:], o_sb[0:64, :],
                                                 rb_sb[:])
                        for h in range(HL):
                            nc.gpsimd.dma_start(
                                inb[b].ap()[2 * j:2 * j + 2,
                                            64 * h:64 * (h + 1), :]
                                .rearrange("s p u -> p s u"),
                                ofin[:, 512 * h:512 * (h + 1)].rearrange(
                                    "p (s u) -> p s u", s=2))

                    pending.append(norm_tail)
                while emitted < len(units):
                    units[emitted]()
                    emitted += 1
                flush_pending()

            def emit_a2a(b):
                if sim:
                    # stand-in with comparable cost for the cost-model sim
                    nc.sync.dma_start(outb[b].ap(), inb[b].ap())
                else:
                    nc.gpsimd.collective_compute(
                        "AllToAll", mybir.AluOpType.bypass,
                        replica_groups=[list(range(NCORES))],
                        ins=[inb[b].ap().opt()], outs=[outb[b].ap().opt()],
                    )

            # prologue: batch-0 qkv runs un-interleaved
            for u in qkv_units(0):
                u()
            for b in range(B):
                units = qkv_units(b + 1) if b + 1 < B else []
                units = interleave(units, outproj_units(b - 1) if b else [])
                emit_attn(b, units)
                emit_a2a(b)
                emit_ot_load(b)
            for u in outproj_units(B - 1):
                u()
    nc.compile()
    return nc


def _get_nc():
    if "nc" not in _CACHE:
        _CACHE["nc"] = _build()
    return _CACHE["nc"]


def kernel(x, W_qkv, b_qkv, W_proj, b_proj):
    x = np.asarray(x, dtype=np.float32)
    W_qkv = np.asarray(W_qkv, dtype=np.float32)
    b_qkv = np.asarray(b_qkv, dtype=np.float32)
    W_proj = np.asarray(W_proj, dtype=np.float32)
    b_proj = np.asarray(b_proj, dtype=np.float32)

    BF = ml_dtypes.bfloat16
    scale = 1.0 / np.sqrt(HD)
    xt = np.ascontiguousarray(x.reshape(B * T, C).T).astype(BF)  # [C, B*T]
    wp = np.ascontiguousarray(W_proj).astype(BF)                 # [C, C]
    bp = np.ascontiguousarray(b_proj.reshape(NKC, 128).T)        # [128, 8]

    cn = np.zeros((128, CN_W), dtype=BF)
    cn[:, CN_ID:CN_ID + 128] = np.eye(128, dtype=np.float32)
    tril = (np.arange(128)[None, :] >= np.arange(128)[:, None])
    cn[:, CN_MASK:CN_MASK + 128] = tril.astype(np.float32)
    cn[:, CN_MASK + 128:CN_MASK + 256] = tril.astype(np.float32)
    cn[:, CN_ONES:CN_ONES + 32] = 1.0
    cn[0, CN_ONESR:CN_ONESR + 64] = 1.0

    qw = W_qkv[:, 0:C]
    kw = W_qkv[:, C:2 * C]
    vw = W_qkv[:, 2 * C:3 * C]
    qb, kb, vb = b_qkv[0:C], b_qkv[C:2 * C], b_qkv[2 * C:3 * C]

    in_maps = []
    for c in range(NCORES):
        cols = slice(2 * c * HD, (2 * c + 2) * HD)  # this core's 128 dims
        bq = np.stack([qb[cols] * scale, kb[cols], vb[cols]], axis=1)  # [128,3]
        in_maps.append({
            "xt": xt,
            "wq": np.ascontiguousarray(qw[:, cols] * scale).astype(BF),
            "wk": np.ascontiguousarray(kw[:, cols]).astype(BF),
            "wv": np.ascontiguousarray(vw[:, cols]).astype(BF),
            "wp": wp,
            "cn": cn,
            "bqkv": np.ascontiguousarray(bq),
            "bp": bp,
        })

    nc = _get_nc()
    _CACHE["last_in_maps"] = in_maps
    res = run_bass_kernel_spmd(nc, in_maps, core_ids=list(range(NCORES)))

    # outp[c]: [C, B*PIECE] (cols: b-major, then 256 tokens of piece c)
    allo = np.stack([np.asarray(res.results[c]["outp"], dtype=np.float32)
                     for c in range(NCORES)])
    allo = allo.reshape(NCORES, C, B, PIECE)       # [c, ch, b, u]
    out = allo.transpose(2, 0, 3, 1).reshape(B, T, C)
    return np.ascontiguousarray(out)


# revision 10
# speedup vs baseline: 1.3466x; 1.0476x over previous
"""Causal self-attention (B=4, T=2048, C=1024, H=16) on 8 trn2 NeuronCores.

Sharding: head-pair parallel. Core c owns heads {2c, 2c+1} for all 4 batches.
 - host: x pre-transposed to xT [C, B*T] bf16; W_qkv pre-sliced per core into
   wq/wk/wv [C, 128] bf16 (softmax scale folded into wq), W_proj bf16,
   f32 biases, and a packed bf16 constants tensor (identity, causal tril
   mask, ones) so the kernel needs no GpSimd compute ucode.
 - software pipeline: the attention tile loop of batch b doubles as the
   scheduler for everything else. qkv projections of batch b+1, the V
   transposes of b+1, and the out-projection of b-1 are emitted as small
   self-contained "filler units" between attention tiles, so the PE stays
   busy instead of pacing on ScalarE's exp. x tiles are prefetched two
   batches ahead ([128, 2048] per contraction chunk, one DMA each).
 - attention in S^T orientation: S^T[tk, tq] = kT.T@qT bf16 tiles
   [128, 512], diagonal tiles first. P^T = exp(S^T) on ScalarE (PSUM->SBUF
   bf16); the 128x128 causal wedge of diagonal tiles is zeroed on DVE by
   multiplying with a host-shipped 0/1 tril mask (exp of unmasked future
   logits is finite so this is safe). O-matmul lhsT = [v_h | ones] (M=65)
   gives O^T and the softmax denominator in one pass. Normalize: only the
   PSUM->SBUF copy happens at chunk end; reciprocal + ones-broadcast matmul
   + multiply + stores are deferred into the next chunk's tile loop.
 - per-batch AllToAll (512 KB/rank bf16) reshards O^T head->token shards.
   SBUF->DRAM stores go through the (otherwise idle) GpSimd DGE queue so
   they never head-of-line block the sync DGE queue that feeds loads and
   the collective. Column-parallel out-projection, bf16 out^T, host
   upcasts and reassembles.
"""
import numpy as np
import ml_dtypes
import concourse.bacc as bacc
import concourse.mybir as mybir
import concourse.tile as tile
from concourse.bass_utils import run_bass_kernel_spmd

F32 = mybir.dt.float32
BF16 = mybir.dt.bfloat16
Exp = mybir.ActivationFunctionType.Exp

NCORES = 8
B, T, C, H = 4, 2048, 1024, 16
HD = C // H          # 64
HL = H // NCORES     # 2 heads per core
D2 = HL * HD         # 128 rows of local head-pair dims
TB = T               # tokens per batch
NKC = C // 128       # 8 contraction chunks
NCH = TB // 512      # 4 tq chunks per batch
NTK = TB // 128      # 16 tk tiles per batch
PIECE = TB // NCORES  # 256 tokens per (batch, core) piece after AllToAll

# packed constants layout (bf16, [128, 480]):
#   cols 0:128  identity; 128:384 tril mask x2 heads; 384:416 ones (vh);
#   row 0 cols 416:480 ones (rb lhsT)
CN_ID = 0
CN_MASK = 128
CN_ONES = 384
CN_ONESR = 416
CN_W = 480

_CACHE = {}


def _build(sim=False):
    nc = bacc.Bacc("TRN2", target_bir_lowering=False, debug=False,
                   num_devices=1 if sim else NCORES)
    xt = nc.dram_tensor("xt", [C, B * T], BF16, kind="ExternalInput").ap()
    wq = nc.dram_tensor("wq", [C, D2], BF16, kind="ExternalInput").ap()
    wk = nc.dram_tensor("wk", [C, D2], BF16, kind="ExternalInput").ap()
    wv = nc.dram_tensor("wv", [C, D2], BF16, kind="ExternalInput").ap()
    wp = nc.dram_tensor("wp", [C, C], BF16, kind="ExternalInput").ap()
    cn = nc.dram_tensor("cn", [128, CN_W], BF16, kind="ExternalInput").ap()
    bqkv = nc.dram_tensor("bqkv", [D2, 3], F32, kind="ExternalInput").ap()
    bp = nc.dram_tensor("bp", [128, NKC], F32, kind="ExternalInput").ap()
    outp = nc.dram_tensor("outp", [C, B * PIECE], BF16,
                          kind="ExternalOutput").ap()

    inb = [nc.dram_tensor(f"inb{b}", [NCORES, D2, PIECE], BF16)
           for b in range(B)]
    outb = [nc.dram_tensor(f"outb{b}", [NCORES, D2, PIECE], BF16)
            for b in range(B)]

    with tile.TileContext(nc) as tc:
        with (
            tc.tile_pool(name="const", bufs=1) as cpool,
            tc.tile_pool(name="w", bufs=1) as wpool,
            tc.tile_pool(name="xt", bufs=16) as xpool,
            tc.tile_pool(name="qk", bufs=2) as qkpool,
            tc.tile_pool(name="vstg", bufs=1) as vstgpool,
            tc.tile_pool(name="vh", bufs=2) as vhpool,
            tc.tile_pool(name="pt", bufs=5) as ptpool,
            tc.tile_pool(name="small", bufs=3) as smallpool,
            tc.tile_pool(name="ofin", bufs=3) as ofinpool,
            tc.tile_pool(name="proj", bufs=3) as projpool,
            tc.tile_pool(name="otp", bufs=2) as otpool,
            tc.tile_pool(name="mm", bufs=1, space="PSUM") as mmps,
            tc.tile_pool(name="s", bufs=2, space="PSUM") as sps,
            tc.tile_pool(name="o", bufs=1, space="PSUM") as ops,
        ):
            # ---- constants / weights, ordered so batch-0 qkv starts asap
            wq_sb = wpool.tile([128, NKC, D2], BF16)
            wk_sb = wpool.tile([128, NKC, D2], BF16)
            wv_sb = wpool.tile([128, NKC, D2], BF16)
            nc.sync.dma_start(
                wq_sb[:], wq.rearrange("(kc p) m -> p kc m", p=128))

            xts = {}

            def emit_xt_loads(b, striped=False):
                xts[b] = [xpool.tile([128, TB], BF16, name="xtile")
                          for _ in range(NKC)]
                if striped:
                    # two half-tile DMAs per chunk so batch-0 compute can
                    # start after the first 8 arrive
                    for half in range(2):
                        for kc in range(NKC):
                            nc.sync.dma_start(
                                xts[b][kc][:, 1024 * half:1024 * (half + 1)],
                                xt[128 * kc:128 * (kc + 1),
                                   b * TB + 1024 * half:
                                   b * TB + 1024 * (half + 1)])
                else:
                    for kc in range(NKC):
                        nc.sync.dma_start(
                            xts[b][kc][:],
                            xt[128 * kc:128 * (kc + 1), b * TB:(b + 1) * TB])

            cn_sb = cpool.tile([128, CN_W], BF16)
            nc.sync.dma_start(cn_sb[:], cn)
            emit_xt_loads(0, striped=True)
            nc.sync.dma_start(
                wk_sb[:], wk.rearrange("(kc p) m -> p kc m", p=128))
            bqkv_sb = cpool.tile([D2, 3], F32)
            nc.sync.dma_start(bqkv_sb[:], bqkv)
            nc.sync.dma_start(
                wv_sb[:], wv.rearrange("(kc p) m -> p kc m", p=128))
            identb = cn_sb[:, CN_ID:CN_ID + 128]
            maskb = cn_sb[:, CN_MASK:CN_MASK + 256].rearrange(
                "p (g c) -> p g c", g=HL)
            onesb = cn_sb[:, CN_ONES:CN_ONES + 32]
            onesr = cn_sb[0:1, CN_ONESR:CN_ONESR + 64]

            emit_xt_loads(1)
            wp_sb = wpool.tile([128, NKC, C], BF16)
            for g in range(4):
                nc.sync.dma_start(
                    wp_sb[:, 2 * g:2 * (g + 1), :],
                    wp[256 * g:256 * (g + 1), :].rearrange(
                        "(two p) m -> p two m", p=128))
            bp_sb = cpool.tile([128, NKC], F32)
            nc.sync.dma_start(bp_sb[:], bp)

            qT = {}
            kT = {}
            vh = {}
            vT = {}

            def qkv_units(b):
                """12 qkv matmul groups + 4 v-transpose groups for batch b."""
                qT[b] = qkpool.tile([D2, TB], BF16, tag="qT", name="qT")
                kT[b] = qkpool.tile([D2, TB], BF16, tag="kT", name="kT")
                vT[b] = vstgpool.tile([D2, TB], BF16, name="vT")
                units = []

                def qkv_group(n, col, w_sb, dst, b=b):
                    def emit():
                        ps = mmps.tile([128, 512], F32, tag="qps",
                                       name="ps")
                        for kc in range(NKC):
                            nc.tensor.matmul(
                                ps[:], w_sb[:, kc, :],
                                xts[b][kc][:, 512 * n:512 * (n + 1)],
                                start=(kc == 0), stop=(kc == NKC - 1))
                        with nc.allow_low_precision(reason="bf16 qkv"):
                            nc.vector.tensor_scalar_add(
                                dst[:, 512 * n:512 * (n + 1)], ps[:],
                                bqkv_sb[:, col:col + 1])
                        if n == NCH - 1 and col == 2:
                            del xts[b]
                    return emit

                for n in range(NCH):
                    for col, (w_sb, dst) in enumerate(
                            ((wq_sb, qT[b]), (wk_sb, kT[b]),
                             (wv_sb, vT[b]))):
                        units.append(qkv_group(n, col, w_sb, dst))

                def vt_group(tk0, b=b):
                    def emit():
                        # layout: vh[:, h*(NTK*65) + tk*65 + (0:64)] = v tile
                        if tk0 == 0:
                            vh[b] = vhpool.tile([128, HL * NTK * 65], BF16,
                                                tag="vh", name="vh")
                            with nc.allow_low_precision(reason="ones"):
                                nc.vector.tensor_copy(vh[b][:, 64::65],
                                                      onesb[:])
                        vt_ps = mmps.tile([128, 512], BF16, tag="vtps")
                        for i in range(4):
                            nc.tensor.transpose(
                                vt_ps[:, 128 * i:128 * (i + 1)],
                                vT[b][:, 128 * (tk0 + i):
                                      128 * (tk0 + i + 1)],
                                identb)
                        with nc.allow_low_precision(reason="bf16 v"):
                            nc.vector.tensor_copy(
                                vh[b][:].rearrange(
                                    "p (h tk c) -> p h tk c",
                                    h=HL, tk=NTK)[:, :, tk0:tk0 + 4, 0:64],
                                vt_ps[:].rearrange(
                                    "p (i h c) -> p h i c", i=4, h=HL))
                    return emit

                for tk0 in range(0, NTK, 4):
                    units.append(vt_group(tk0))
                return units

            ot = {}

            def emit_ot_load(b):
                ot[b] = otpool.tile([128, NCORES, PIECE], BF16, name="ot")
                nc.sync.dma_start(
                    ot[b][:], outb[b].ap().rearrange("s p u -> p s u"))

            def outproj_units(b):
                """4 out-projection column-pair groups for batch b."""
                units = []

                def op_group(mc2, b=b):
                    def emit():
                        osb = projpool.tile([128, 2, PIECE], BF16,
                                            tag="osb", name="osb")
                        for sub in range(2):
                            mcol = 2 * mc2 + sub
                            pp = mmps.tile([128, 512], F32, tag="qps",
                                           name="pp")
                            for s8 in range(NCORES):
                                nc.tensor.matmul(
                                    pp[:, 0:PIECE],
                                    wp_sb[:, s8,
                                          128 * mcol:128 * (mcol + 1)],
                                    ot[b][:, s8, :], start=(s8 == 0),
                                    stop=(s8 == NCORES - 1))
                            with nc.allow_low_precision(reason="bf16 out"):
                                nc.vector.tensor_scalar_add(
                                    osb[:, sub, :], pp[:, 0:PIECE],
                                    bp_sb[:, mcol:mcol + 1])
                        nc.gpsimd.dma_start(
                            outp[256 * mc2:256 * (mc2 + 1),
                                 PIECE * b:PIECE * (b + 1)].rearrange(
                                     "(two p) u -> p two u", p=128),
                            osb[:])
                    return emit

                for mc2 in range(NKC // 2):
                    units.append(op_group(mc2))
                return units

            def interleave(a, bl):
                """a with elements of bl spread through (a leads)."""
                if not bl:
                    return list(a)
                out = []
                step = max(1, len(a) // (len(bl) + 1))
                bi = 0
                for i, u in enumerate(a):
                    out.append(u)
                    if bi < len(bl) and (i + 1) % step == 0 and i >= 2:
                        out.append(bl[bi])
                        bi += 1
                out.extend(bl[bi:])
                return out

            pending = []

            def flush_pending():
                while pending:
                    pending.pop(0)()

            def emit_attn(b, units, tail_fn=None):
                if b + 2 < B:
                    emit_xt_loads(b + 2)
                n_tiles = sum(4 * j + 4 for j in range(NCH))
                done = 0
                emitted = 0
                for j in range(NCH):
                    o_ps = ops.tile([65, 1024], F32, tag="o")
                    # diagonal tiles first: m=0 is full width and opens the
                    # O accumulation; wedge masks pipeline behind exp
                    seq = list(range(4 * j, 4 * j + 4)) + list(range(4 * j))
                    for idx, tk in enumerate(seq):
                        m = tk - 4 * j
                        z = 128 * m if m >= 0 else 0
                        s_ps = sps.tile([128, 1024], F32, tag="s_ps",
                                        name="s_ps")
                        for h in range(HL):
                            nc.tensor.matmul(
                                s_ps[:, 512 * h + z:512 * (h + 1)],
                                kT[b][64 * h:64 * (h + 1),
                                      128 * tk:128 * (tk + 1)],
                                qT[b][64 * h:64 * (h + 1),
                                      512 * j + z:512 * (j + 1)],
                                start=True, stop=True)
                        pt = ptpool.tile([128, 1024], BF16, tag="pt")
                        if z:
                            exp_src = s_ps[:].rearrange(
                                "p (g c) -> p g c", g=2)[:, :, z:]
                            exp_dst = pt[:].rearrange(
                                "p (g c) -> p g c", g=2)[:, :, z:]
                            nc.scalar.activation(exp_dst, exp_src, Exp)
                        else:
                            nc.scalar.activation(pt[:], s_ps[:], Exp)
                        if m >= 0:
                            # zero the causal wedge: keep where tq_l >= tk_l
                            ptv = pt[:].rearrange(
                                "p (g c) -> p g c", g=2)[:, :, z:z + 128]
                            with nc.allow_low_precision(reason="mask"):
                                nc.vector.tensor_mul(ptv, ptv, maskb)
                        for h in range(HL):
                            nc.tensor.matmul(
                                o_ps[0:65, 512 * h + z:512 * (h + 1)],
                                vh[b][:, (h * NTK + tk) * 65:
                                      (h * NTK + tk + 1) * 65],
                                pt[:, 512 * h + z:512 * (h + 1)],
                                start=(idx == 0), stop=(idx == len(seq) - 1))
                        done += 1
                        if idx == 2:
                            flush_pending()
                            if j == 0 and tail_fn is not None:
                                tail_fn()
                                tail_fn = None
                        # pace filler so it finishes ~4 tiles before the end
                        target = min(len(units),
                                     done * len(units) // max(1, n_tiles - 4))
                        while emitted < target:
                            units[emitted]()
                            emitted += 1

                    # normalize: only the PSUM->SBUF copy now; the rest is
                    # deferred so it never head-of-line blocks PE
                    o_sb = smallpool.tile([65, 1024], BF16, tag="osb2",
                                          name="o_sb")
                    with nc.allow_low_precision(reason="bf16 O"):
                        nc.vector.tensor_copy(o_sb[:], o_ps[:])

                    def norm_tail(b=b, j=j, o_sb=o_sb):
                        r_sb = smallpool.tile([1, 1024], BF16, tag="r",
                                              name="r_sb")
                        with nc.allow_low_precision(reason="denom"):
                            nc.vector.reciprocal(r_sb[:], o_sb[64:65, :])
                        rb_ps = sps.tile([128, 1024], F32, tag="s_ps",
                                         name="rb_ps")
                        nc.tensor.matmul(rb_ps[0:64, :], onesr, r_sb[:],
                                         start=True, stop=True)
                        rb_sb = smallpool.tile([64, 1024], BF16, tag="rb",
                                               name="rb_sb")
                        with nc.allow_low_precision(reason="bf16 rb"):
                            nc.vector.tensor_copy(rb_sb[:], rb_ps[0:64, :])
                        ofin = ofinpool.tile([64, 1024], BF16, name="ofin")
                        with nc.allow_low_precision(reason="bf16 O"):
                            nc.vector.tensor_mul(ofin[:], o_sb[0:64, :],
                                                 rb_sb[:])
                        for h in range(HL):
                            nc.gpsimd.dma_start(
                                inb[b].ap()[2 * j:2 * j + 2,
                                            64 * h:64 * (h + 1), :]
                                .rearrange("s p u -> p s u"),
                                ofin[:, 512 * h:512 * (h + 1)].rearrange(
                                    "p (s u) -> p s u", s=2))

                    pending.append(norm_tail)
                while emitted < len(units):
                    units[emitted]()
                    emitted += 1

            def emit_a2a(b):
                if sim:
                    # stand-in with comparable cost for the cost-model sim
                    nc.sync.dma_start(outb[b].ap(), inb[b].ap())
                else:
                    nc.gpsimd.collective_compute(
                        "AllToAll", mybir.AluOpType.bypass,
                        replica_groups=[list(range(NCORES))],
                        ins=[inb[b].ap().opt()], outs=[outb[b].ap().opt()],
                    )

            # prologue: batch-0 qkv runs un-interleaved
            for u in qkv_units(0):
                u()
            for b in range(B):
                units = qkv_units(b + 1) if b + 1 < B else []
                units = interleave(units, outproj_units(b - 1) if b else [])

                def tail_fn(bb=b - 1):
                    emit_a2a(bb)
                    emit_ot_load(bb)

                emit_attn(b, units, tail_fn if b else None)
            flush_pending()
            emit_a2a(B - 1)
            emit_ot_load(B - 1)
            for u in outproj_units(B - 1):
                u()
    nc.compile()
    return nc


def _get_nc():
    if "nc" not in _CACHE:
        _CACHE["nc"] = _build()
    return _CACHE["nc"]


def kernel(x, W_qkv, b_qkv, W_proj, b_proj):
    x = np.asarray(x, dtype=np.float32)
    W_qkv = np.asarray(W_qkv, dtype=np.float32)
    b_qkv = np.asarray(b_qkv, dtype=np.float32)
    W_proj = np.asarray(W_proj, dtype=np.float32)
    b_proj = np.asarray(b_proj, dtype=np.float32)

    BF = ml_dtypes.bfloat16
    scale = 1.0 / np.sqrt(HD)
    xt = np.ascontiguousarray(x.reshape(B * T, C).T).astype(BF)  # [C, B*T]
    wp = np.ascontiguousarray(W_proj).astype(BF)                 # [C, C]
    bp = np.ascontiguousarray(b_proj.reshape(NKC, 128).T)        # [128, 8]

    cn = np.zeros((128, CN_W), dtype=BF)
    cn[:, CN_ID:CN_ID + 128] = np.eye(128, dtype=np.float32)
    tril = (np.arange(128)[None, :] >= np.arange(128)[:, None])
    cn[:, CN_MASK:CN_MASK + 128] = tril.astype(np.float32)
    cn[:, CN_MASK + 128:CN_MASK + 256] = tril.astype(np.float32)
    cn[:, CN_ONES:CN_ONES + 32] = 1.0
    cn[0, CN_ONESR:CN_ONESR + 64] = 1.0

    qw = W_qkv[:, 0:C]
    kw = W_qkv[:, C:2 * C]
    vw = W_qkv[:, 2 * C:3 * C]
    qb, kb, vb = b_qkv[0:C], b_qkv[C:2 * C], b_qkv[2 * C:3 * C]

    in_maps = []
    for c in range(NCORES):
        cols = slice(2 * c * HD, (2 * c + 2) * HD)  # this core's 128 dims
        bq = np.stack([qb[cols] * scale, kb[cols], vb[cols]], axis=1)  # [128,3]
        in_maps.append({
            "xt": xt,
            "wq": np.ascontiguousarray(qw[:, cols] * scale).astype(BF),
            "wk": np.ascontiguousarray(kw[:, cols]).astype(BF),
            "wv": np.ascontiguousarray(vw[:, cols]).astype(BF),
            "wp": wp,
            "cn": cn,
            "bqkv": np.ascontiguousarray(bq),
            "bp": bp,
        })

    nc = _get_nc()
    _CACHE["last_in_maps"] = in_maps
    res = run_bass_kernel_spmd(nc, in_maps, core_ids=list(range(NCORES)))

    # outp[c]: [C, B*PIECE] (cols: b-major, then 256 tokens of piece c)
    allo = np.stack([np.asarray(res.results[c]["outp"], dtype=np.float32)
                     for c in range(NCORES)])
    allo = allo.reshape(NCORES, C, B, PIECE)       # [c, ch, b, u]
    out = allo.transpose(2, 0, 3, 1).reshape(B, T, C)
    return np.ascontiguousarray(out)
